# revision 2
# baseline (speedup 1.0000x reference)
"""MLA q/k/v projection kernel for Trainium2 (8 NeuronCores, token-data-parallel).

Self-contained: hardcodes the problem shapes from nn_MLA_81106162418389.
  hidden_state [2, 4096, 2048] f32 -> out [2, 16, 4096, 512] f32
Strategy: shard the 8192 tokens over 8 cores (1024 each); replicate weights.
All matmul operands are bf16 (halves load DMA vs f32; PE rate identical).

Per-core pipeline over 8 chunks of 128 tokens:
  mm1  [tok, 1344] = xT-chunk^T @ [w_qa|w_kva]^T   (PSUM, 3 col tiles)
  LN   stats along the free axis on DVE, normalize on ACT (PSUM->SBUF bf16)
  tr   PE transposes [tok,feat]->[feat,tok] for the up-proj lhsT
  mm2  per 2-head col group; assemble q|k|v (+RoPE) into [tok, 4head, 512]
  DMA  store per 4-head group (contiguous 2KB runs)
PE program order: mm1(0) mm1(1) tr(0) mm2(0) mm1(2) tr(1) mm2(1) ...
"""
import sys
sys.path.insert(0, "/opt/trn_rl_repo")

import numpy as np

import concourse.bass as bass
import concourse.tile as tile
from concourse import bacc, mybir
from concourse import bass2jax

# ---- problem constants ----
HID, QK_NOPE, QK_ROPE, Q_LR, KV_LR, H, V_DIM = 2048, 128, 64, 768, 512, 16, 128
QK_HEAD = QK_NOPE + QK_ROPE           # 192
OUT_C = 2 * QK_HEAD + V_DIM           # 512
B, S = 2, 4096
THETA = 10000.0
EPS = 1e-5

N_CORES = 8
T = (B * S) // N_CORES                # 1024 tokens per core
P = 128
TCN = T // P                          # 8 token chunks
KO = HID // P                         # 16 hid chunks for a-proj
AFE = Q_LR + KV_LR + QK_ROPE          # 1344 a-proj out cols [q|kv|rope]
ROQ = Q_LR // P                       # 6 feat chunks for q up-proj
ROKV = KV_LR // P                     # 4 feat chunks for kv up-proj
QW = H * QK_HEAD                      # 3072 q up-proj cols
KW = H * (QK_NOPE + V_DIM)            # 4096 kv up-proj cols

F32 = mybir.dt.float32
R32 = mybir.dt.float32r
BF16 = mybir.dt.bfloat16
AF = mybir.ActivationFunctionType
OP = mybir.AluOpType
AX = mybir.AxisListType


def _build(n_repeats=1, has_a_bias=False, has_b_bias=False):
    nc = bacc.Bacc("TRN2", target_bir_lowering=False, debug=False,
                   num_devices=N_CORES)

    x_d = nc.dram_tensor("xp", [P, TCN, KO, P], BF16, kind="ExternalInput").ap()
    wa_d = nc.dram_tensor("wap", [KO, P, AFE], BF16, kind="ExternalInput").ap()
    wqb_d = nc.dram_tensor("wqbp", [ROQ, P, QW], BF16, kind="ExternalInput").ap()
    wkvb_d = nc.dram_tensor("wkvbp", [ROKV, P, KW], BF16,
                            kind="ExternalInput").ap()
    cs_d = nc.dram_tensor("cstab", [T, 2 * QK_ROPE], F32,
                          kind="ExternalInput").ap()
    id_d = nc.dram_tensor("identt", [P, P], R32, kind="ExternalInput").ap()
    ba_d = bqb_d = bkvb_d = None
    if has_a_bias:
        ba_d = nc.dram_tensor("bap", [AFE], F32, kind="ExternalInput").ap()
    if has_b_bias:
        bqb_d = nc.dram_tensor("bqb", [QW], F32, kind="ExternalInput").ap()
        bkvb_d = nc.dram_tensor("bkvb", [KW], F32, kind="ExternalInput").ap()
    out_d = nc.dram_tensor("out", [H, T, OUT_C], F32, kind="ExternalOutput").ap()

    for _ in range(n_repeats):
        _emit_once(nc, x_d, wa_d, wqb_d, wkvb_d, cs_d, id_d,
                   ba_d, bqb_d, bkvb_d, out_d)
    nc.compile()
    return nc


def _emit_once(nc, x_d, wa_d, wqb_d, wkvb_d, cs_d, id_d,
               ba_d, bqb_d, bkvb_d, out_d):
    outT = out_d.rearrange("h t c -> t h c")
    with tile.TileContext(nc) as tc:
        with tc.tile_pool(name="main", bufs=1) as mp, \
             tc.tile_pool(name="psa", bufs=1, space="PSUM") as pa, \
             tc.tile_pool(name="ps2", bufs=1, space="PSUM") as p2:

            # ---- persistent small tiles (cs DMA issued mid-stream) ----
            # cs_sb[:, c, 0:64] = cos; [:, c, 64:96] = -sin[0:32];
            # [:, c, 96:128] = sin[32:64]
            cs_sb = mp.tile([P, TCN, 2 * QK_ROPE], F32)
            eps_bc = mp.tile([P, 1], F32)
            nc.gpsimd.memset(eps_bc[:], EPS)
            ident = mp.tile([P, P], R32)
            nc.sync.dma_start(ident[:], id_d[:])
            # preload the act table containing Sqrt+Square+Identity+Copy so
            # no LoadActFuncSet lands mid-pipeline
            actwarm = mp.tile([1, 1], F32)
            nc.scalar.activation(actwarm[:], eps_bc[0:1, 0:1], AF.Sqrt)

            bias_bc = qb_bc = kvb_bc = None
            if ba_d is not None:
                b1 = mp.tile([1, AFE], F32)
                nc.sync.dma_start(b1[:], ba_d[None, :])
                bias_bc = mp.tile([P, AFE], F32)
                nc.gpsimd.partition_broadcast(bias_bc[:], b1[:])
            if bqb_d is not None:
                b2 = mp.tile([1, QW], F32)
                nc.sync.dma_start(b2[:], bqb_d[None, :])
                qb_bc = mp.tile([P, QW], F32)
                nc.gpsimd.partition_broadcast(qb_bc[:], b2[:])
                b3 = mp.tile([1, KW], F32)
                nc.sync.dma_start(b3[:], bkvb_d[None, :])
                kvb_bc = mp.tile([P, KW], F32)
                nc.gpsimd.partition_broadcast(kvb_bc[:], b3[:])

            # ---- weight + x loads (SP queue order matters) ----
            xt = [None] * TCN

            def load_x(c):
                t = mp.tile([P, KO, P], BF16, tag="xt", bufs=4, name=f"x_{c}")
                if c == 0:
                    nc.sync.dma_start(t[:, 0:KO // 2], x_d[:, c, 0:KO // 2])
                    nc.sync.dma_start(t[:, KO // 2:], x_d[:, c, KO // 2:])
                else:
                    nc.sync.dma_start(t[:], x_d[:, c])
                xt[c] = t

            load_x(0)
            wa_t = []
            for k in range(KO):
                w = mp.tile([P, AFE], BF16, tag="wa", bufs=KO, name=f"wa_{k}")
                nc.sync.dma_start(w[:], wa_d[k])
                wa_t.append(w)
                if k == 10:
                    # tiny, needed by the first rope ops ~25us in
                    nc.sync.dma_start(
                        cs_sb[:], cs_d.rearrange("(tc p) c -> p tc c", p=P))
                if k == 3:
                    load_x(1)
            wqb_t = []
            for ro in range(ROQ):
                w = mp.tile([P, QW], BF16, tag="wqb", bufs=ROQ, name=f"wqb_{ro}")
                wqb_t.append(w)
            wkvb_t = []
            for ro in range(ROKV):
                w = mp.tile([P, KW], BF16, tag="wkvb", bufs=ROKV,
                            name=f"wkvb_{ro}")
                wkvb_t.append(w)
            # column-half loads: groups 0-3 of mm2 only need the first half
            for half in range(2):
                cs = slice(half * QW // 2, (half + 1) * QW // 2)
                for ro in range(ROQ):
                    nc.sync.dma_start(wqb_t[ro][:, cs], wqb_d[ro][:, cs])
                if half == 0:
                    load_x(2)
            for half in range(2):
                cs = slice(half * KW // 2, (half + 1) * KW // 2)
                for ro in range(ROKV):
                    nc.sync.dma_start(wkvb_t[ro][:, cs], wkvb_d[ro][:, cs])
                if half == 0:
                    load_x(3)

            # per-chunk state
            qn = [None] * TCN       # [tok, 768] bf16 normalized q low-rank
            kvn = [None] * TCN      # [tok, 512] bf16 normalized kv low-rank
            kr = [None] * TCN       # [tok, 64] f32 raw k rope
            krot = [None] * TCN     # [tok, 64] f32 roped k
            qkvT = [None] * TCN     # [feat, 10, tok] bf16 transposed lhsT
            psa = [None] * TCN      # 3 psum tiles of mm1

            # mm1 col groups: (psum tile idx, col offset in tile, width)
            CG = ((0, 0, 512), (1, 0, 512), (2, 0, 320))

            def mm1(c):
                a = [pa.tile([P, 512], F32, tag="psa", bufs=6,
                             name=f"a{g}_{c}") for g in range(3)]
                psa[c] = a
                for k in range(KO):
                    for g, (ti, off, w) in enumerate(CG):
                        nc.tensor.matmul(
                            a[ti][:, off:off + w], xt[c][:, k, :],
                            wa_t[k][:, g * 512:g * 512 + w],
                            start=(k == 0), stop=(k == KO - 1))

            def ln(c):
                a0, a1, a2 = psa[c]
                if bias_bc is not None:
                    nc.vector.tensor_tensor(a0[:], a0[:], bias_bc[:, 0:512],
                                            OP.add)
                    nc.vector.tensor_tensor(a1[:], a1[:], bias_bc[:, 512:1024],
                                            OP.add)
                    nc.vector.tensor_tensor(a2[:, 0:320], a2[:, 0:320],
                                            bias_bc[:, 1024:1344], OP.add)
                st = {}
                for nm in ("sq", "skv", "qq", "qkv", "mu_q", "mu_kv", "var_q",
                           "var_kv", "istd_q", "istd_kv", "nmi_q", "nmi_kv",
                           "p1", "p2", "p3", "p4"):
                    st[nm] = mp.tile([P, 1], F32, tag=f"st_{nm}", bufs=2,
                                     name=f"{nm}_{c}")
                scr = mp.tile([P, 512], BF16, tag="scr", bufs=2,
                              name=f"scr_{c}")
                q = mp.tile([P, Q_LR], R32, tag="qn", bufs=2, name=f"qn_{c}")
                kv = mp.tile([P, KV_LR], R32, tag="kvn", bufs=2,
                             name=f"kvn_{c}")
                k_ = mp.tile([P, QK_ROPE], F32, tag="kr", bufs=2,
                             name=f"kr_{c}")

                def finish_stats(which, dim):
                    mu, var = st[f"mu_{which}"], st[f"var_{which}"]
                    istd, nmi = st[f"istd_{which}"], st[f"nmi_{which}"]
                    ssum = st["sq"] if which == "q" else st["skv"]
                    ssq = st["qq"] if which == "q" else st["qkv"]
                    nc.scalar.mul(mu[:], ssum[:], 1.0 / dim)
                    nc.vector.tensor_tensor(var[:], mu[:], mu[:], OP.mult)
                    nc.vector.scalar_tensor_tensor(
                        var[:], ssq[:], 1.0 / dim, var[:], OP.mult, OP.subtract)
                    nc.scalar.activation(istd[:], var[:], AF.Sqrt,
                                         bias=eps_bc[:, 0:1])
                    nc.vector.reciprocal(istd[:], istd[:])
                    nc.vector.scalar_tensor_tensor(
                        nmi[:], mu[:], -1.0, istd[:], OP.mult, OP.mult)

                # ---- q path ----
                nc.vector.tensor_reduce(st["p1"][:], a0[:], AX.X, OP.add)
                nc.vector.tensor_reduce(st["p2"][:], a1[:, 0:256], AX.X, OP.add)
                nc.vector.tensor_tensor(st["sq"][:], st["p1"][:], st["p2"][:],
                                        OP.add)
                nc.scalar.activation(scr[:], a0[:], AF.Square,
                                     accum_out=st["p1"][:])
                nc.scalar.activation(scr[:, 0:256], a1[:, 0:256], AF.Square,
                                     accum_out=st["p2"][:])
                nc.vector.tensor_tensor(st["qq"][:], st["p1"][:], st["p2"][:],
                                        OP.add)
                finish_stats("q", Q_LR)
                nc.scalar.activation(q[:, 0:512], a0[:], AF.Identity,
                                     bias=st["nmi_q"][:, 0:1],
                                     scale=st["istd_q"][:, 0:1])
                nc.scalar.activation(q[:, 512:768], a1[:, 0:256], AF.Identity,
                                     bias=st["nmi_q"][:, 0:1],
                                     scale=st["istd_q"][:, 0:1])
                # ---- kv path ----
                nc.vector.tensor_reduce(st["p3"][:], a1[:, 256:512], AX.X,
                                        OP.add)
                nc.vector.tensor_reduce(st["p4"][:], a2[:, 0:256], AX.X, OP.add)
                nc.vector.tensor_tensor(st["skv"][:], st["p3"][:], st["p4"][:],
                                        OP.add)
                nc.scalar.activation(scr[:, 256:512], a1[:, 256:512],
                                     AF.Square, accum_out=st["p3"][:])
                nc.scalar.activation(scr[:, 0:256], a2[:, 0:256], AF.Square,
                                     accum_out=st["p4"][:])
                nc.vector.tensor_tensor(st["qkv"][:], st["p3"][:], st["p4"][:],
                                        OP.add)
                finish_stats("kv", KV_LR)
                nc.scalar.activation(kv[:, 0:256], a1[:, 256:512], AF.Identity,
                                     bias=st["nmi_kv"][:, 0:1],
                                     scale=st["istd_kv"][:, 0:1])
                nc.scalar.activation(kv[:, 256:512], a2[:, 0:256], AF.Identity,
                                     bias=st["nmi_kv"][:, 0:1],
                                     scale=st["istd_kv"][:, 0:1])
                nc.scalar.copy(k_[:], a2[:, 256:320])
                qn[c], kvn[c], kr[c] = q, kv, k_
                # k rope rotation (DVE, 4 ops; sin first-half pre-negated)
                kro = mp.tile([P, QK_ROPE], F32, tag="krot", bufs=2,
                              name=f"krot_{c}")
                t12 = mp.tile([P, QK_ROPE], F32, tag="kt12", bufs=2,
                              name=f"kt12_{c}")
                cs = cs_sb[:, c]
                nc.vector.tensor_tensor(t12[:, 0:32], k_[:, 32:64],
                                        cs[:, 64:96], OP.mult)
                nc.vector.tensor_tensor(t12[:, 32:64], k_[:, 0:32],
                                        cs[:, 96:128], OP.mult)
                nc.vector.tensor_tensor(kro[:], k_[:], cs[:, 0:64], OP.mult)
                nc.vector.tensor_tensor(kro[:], kro[:], t12[:], OP.add)
                krot[c] = kro

            def transposes(c):
                qt = mp.tile([P, ROQ + ROKV, P], BF16, tag="qkvT", bufs=2,
                             name=f"qkvT_{c}")
                qkvT[c] = qt
                srcs = [qn[c][:, ro * P:(ro + 1) * P] for ro in range(ROQ)] + \
                       [kvn[c][:, ro * P:(ro + 1) * P] for ro in range(ROKV)]
                # reuse the psa bank set that LN(c) just drained
                for b0 in range(0, ROQ + ROKV, 4):
                    n = min(4, ROQ + ROKV - b0)
                    pt = pa.tile([P, 512], R32, tag="psa", bufs=6,
                                 name=f"tr_{c}_{b0}")
                    for j in range(n):
                        nc.tensor.matmul(pt[:, j * P:(j + 1) * P],
                                         srcs[b0 + j], ident[:],
                                         is_transpose=True,
                                         skip_group_check=True)
                    nc.scalar.copy(
                        qt[:, b0:b0 + n, :],
                        pt[:, 0:n * P].rearrange("p (j t) -> p j t", t=P))

            def mm2(c, q_first=False):
                tsl = slice(c * P, (c + 1) * P)
                obs = [mp.tile([P, 4, OUT_C], F32, tag="ob", bufs=6,
                               name=f"ob_{c}_{t4}") for t4 in range(4)]

                def qgroup(g):
                    ob = obs[g // 2]
                    hh = 2 * (g % 2)
                    pq = p2.tile([P, 512], F32, tag="ps2", bufs=2,
                                 name=f"pq_{c}_{g}")
                    for ro in range(ROQ):
                        nc.tensor.matmul(
                            pq[:, 0:384], qkvT[c][:, ro, :],
                            wqb_t[ro][:, g * 384:(g + 1) * 384],
                            start=(ro == 0), stop=(ro == ROQ - 1))
                    if qb_bc is not None:
                        nc.vector.tensor_tensor(
                            pq[:, 0:384], pq[:, 0:384],
                            qb_bc[:, g * 384:(g + 1) * 384], OP.add)
                    pqv = pq[:, 0:384].rearrange("p (h c) -> p h c", c=192)
                    # copy the whole 192-col slice so the PSUM tile frees
                    # after one ACT op; rope runs on SBUF off the ps2 path
                    nc.scalar.copy(ob[:, hh:hh + 2, 0:QK_HEAD], pqv[:])
                    orp = ob[:, hh:hh + 2, QK_NOPE:QK_HEAD]
                    cb = cs_sb[:, c:c + 1, 0:64].to_broadcast([P, 2, QK_ROPE])
                    s1 = cs_sb[:, c:c + 1, 64:96].to_broadcast([P, 2, 32])
                    s2 = cs_sb[:, c:c + 1, 96:128].to_broadcast([P, 2, 32])
                    t12 = mp.tile([P, 2, QK_ROPE], F32, tag="qt12", bufs=3,
                                  name=f"qt12_{c}_{g}")
                    nc.vector.tensor_tensor(t12[:, :, 0:32],
                                            orp[:, :, 32:64], s1[:], OP.mult)
                    nc.vector.tensor_tensor(t12[:, :, 32:64],
                                            orp[:, :, 0:32], s2[:], OP.mult)
                    nc.vector.tensor_tensor(orp[:], orp[:], cb[:], OP.mult)
                    nc.vector.tensor_tensor(orp[:], orp[:], t12[:], OP.add)

                def kvgroup(g):
                    ob = obs[g // 2]
                    hh = 2 * (g % 2)
                    pk = p2.tile([P, 512], F32, tag="ps2", bufs=2,
                                 name=f"pk_{c}_{g}")
                    for ro in range(ROKV):
                        nc.tensor.matmul(
                            pk[:], qkvT[c][:, ROQ + ro, :],
                            wkvb_t[ro][:, g * 512:(g + 1) * 512],
                            start=(ro == 0), stop=(ro == ROKV - 1))
                    if kvb_bc is not None:
                        nc.vector.tensor_tensor(
                            pk[:], pk[:],
                            kvb_bc[:, g * 512:(g + 1) * 512], OP.add)
                    pkv = pk.rearrange("p (h c) -> p h c", c=256)
                    nc.scalar.copy(
                        ob[:, hh:hh + 2, QK_HEAD:QK_HEAD + QK_NOPE],
                        pkv[:, :, 0:QK_NOPE])
                    nc.vector.tensor_copy(
                        ob[:, hh:hh + 2, QK_HEAD + QK_HEAD:OUT_C],
                        pkv[:, :, QK_NOPE:256])

                def krot_bcast(t4):
                    nc.scalar.copy(
                        obs[t4][:, :, QK_HEAD + QK_NOPE:2 * QK_HEAD],
                        krot[c][:, None, :].to_broadcast([P, 4, QK_ROPE]))

                def finish(t4):
                    ob = obs[t4]
                    h0 = 4 * t4
                    if c >= TCN - 2:
                        # tail: column-split stores -- the q third fires as
                        # soon as the q groups finish, only kv cols wait
                        for hh in (0, 2):
                            hs = slice(h0 + hh, h0 + hh + 2)
                            nc.sync.dma_start(outT[tsl, hs, 0:QK_HEAD],
                                              ob[:, hh:hh + 2, 0:QK_HEAD])
                            nc.sync.dma_start(outT[tsl, hs, QK_HEAD:OUT_C],
                                              ob[:, hh:hh + 2, QK_HEAD:OUT_C])
                    else:
                        nc.sync.dma_start(outT[tsl, h0:h0 + 4, :], ob[:])

                if q_first:
                    for g in range(8):
                        qgroup(g)
                        if g % 2 == 1:
                            krot_bcast(g // 2)
                    for g in range(8):
                        kvgroup(g)
                        if g % 2 == 1:
                            finish(g // 2)
                else:
                    for t4 in range(4):
                        qgroup(2 * t4)
                        qgroup(2 * t4 + 1)
                        krot_bcast(t4)
                        kvgroup(2 * t4)
                        kvgroup(2 * t4 + 1)
                        finish(t4)

            # ---- software pipeline ----
            mm1(0)
            ln(0)
            mm1(1)
            ln(1)
            for c in range(TCN):
                transposes(c)
                if c + 2 < TCN:
                    if 4 <= c + 4 < TCN:
                        load_x(c + 4)
                    mm1(c + 2)
                    ln(c + 2)
                mm2(c, q_first=(c < 2))


# ------------------------- host side -------------------------

NP_BF16 = mybir.dt.np(BF16)


def _rope_tables(s0):
    pos = np.arange(s0, s0 + T, dtype=np.float64)
    inv = 1.0 / THETA ** (np.arange(0, QK_ROPE, 2, dtype=np.float64) / QK_ROPE)
    fr = pos[:, None] * inv[None, :]
    cos = np.concatenate([np.cos(fr), np.cos(fr)], axis=1)
    sin = np.concatenate([np.sin(fr), np.sin(fr)], axis=1)
    # packed: [cos | -sin[0:32] | sin[32:64]]
    return np.concatenate([cos, -sin[:, 0:32], sin[:, 32:64]],
                          axis=1).astype(np.float32)


_prog_cache = {}


def prepare(hidden_state, w_qa, b_qa, g_qa_ln, b_qa_ln, w_qb, b_qb,
            w_kva, b_kva, g_kva_ln, b_kva_ln, w_kvb, b_kvb):
    """Preprocess inputs -> ((has_a, has_b), per-core in_maps)."""
    hidden_state = np.asarray(hidden_state, dtype=np.float32)
    w_qa = np.asarray(w_qa, dtype=np.float32)
    w_qb = np.asarray(w_qb, dtype=np.float32)
    w_kva = np.asarray(w_kva, dtype=np.float32)
    w_kvb = np.asarray(w_kvb, dtype=np.float32)
    b_qa = np.asarray(b_qa, dtype=np.float32)
    b_kva = np.asarray(b_kva, dtype=np.float32)
    g_qa_ln = np.asarray(g_qa_ln, dtype=np.float32)
    b_qa_ln = np.asarray(b_qa_ln, dtype=np.float32)
    g_kva_ln = np.asarray(g_kva_ln, dtype=np.float32)
    b_kva_ln = np.asarray(b_kva_ln, dtype=np.float32)
    b_qb = np.asarray(b_qb, dtype=np.float32)
    b_kvb = np.asarray(b_kvb, dtype=np.float32)

    # [q | kv | rope] a-projection, concat along out features
    wa_cat = np.concatenate([w_qa, w_kva], axis=0)          # [1344, 2048]
    wa_p = np.ascontiguousarray(wa_cat.T).reshape(KO, P, AFE).astype(NP_BF16)
    wqb_p = np.ascontiguousarray(
        (w_qb * g_qa_ln[None, :]).T).reshape(ROQ, P, QW).astype(NP_BF16)
    wkvb_p = np.ascontiguousarray(
        (w_kvb * g_kva_ln[None, :]).T).reshape(ROKV, P, KW).astype(NP_BF16)
    bias_a = np.concatenate([b_qa, b_kva]).astype(np.float32)
    bqb_eff = (b_qb + w_qb @ b_qa_ln).astype(np.float32)
    bkvb_eff = (b_kvb + w_kvb @ b_kva_ln).astype(np.float32)

    has_a = bool(np.any(bias_a))
    has_b = bool(np.any(bqb_eff)) or bool(np.any(bkvb_eff))

    flat = hidden_state.reshape(B * S, HID)
    in_maps = []
    for c in range(N_CORES):
        tok0 = c * T
        s0 = tok0 % S
        cstab = _rope_tables(s0)
        xc = flat[tok0:tok0 + T]                            # [1024, 2048]
        # x_d[p, tc, k, t] = x[tc*128+t, k*128+p]
        xp = np.ascontiguousarray(
            xc.reshape(TCN, P, KO, P).transpose(3, 0, 2, 1)).astype(NP_BF16)
        m = {
            "xp": xp, "wap": wa_p, "wqbp": wqb_p, "wkvbp": wkvb_p,
            "cstab": cstab, "identt": np.eye(P, dtype=np.float32),
        }
        if has_a:
            m["bap"] = bias_a
        if has_b:
            m["bqb"] = bqb_eff
            m["bkvb"] = bkvb_eff
        in_maps.append(m)
    return (has_a, has_b), in_maps


def kernel(**inputs):
    key, in_maps = prepare(**inputs)
    if key not in _prog_cache:
        _prog_cache[key] = _build(1, *key)
    nc = _prog_cache[key]

    res = bass2jax.run_bass_via_pjrt(nc, in_maps, n_cores=N_CORES)

    out = np.empty((B, H, S, OUT_C), np.float32)
    for c in range(N_CORES):
        tok0 = c * T
        b = tok0 // S
        s0 = tok0 % S
        out[b, :, s0:s0 + T, :] = res[c]["out"]
    return out
